# revision 6
# baseline (speedup 1.0000x reference)
"""Masked attention-aggregator kernel for Trainium2 (8 NeuronCores, SPMD).

Reference computation (B=16, N=2048, D=128, DQ=64), all fp32:
    q = x @ Wq.T + bq                      [B, N, DQ]
    k = x @ Wk.T + bk                      [B, N, DQ]
    s = (k @ q.T) / sqrt(DQ)               [B, N, N]   (s[b,n,m] = k[n].q[m])
    w = softmax(s + (mask[m]>0 ? 0 : -1e9), axis=m)
    out = w @ x                            [B, N, D]

Strategy: data-parallel over batch (2 batches per core).  Per batch, a
flash-style streaming attention that never materializes [N, N] in HBM:

  * The masked axis (m) indexes the *queries* side.  Masked m-columns get
    softmax weight exactly 0, so on the host we COMPACT the m axis: gather
    the unmasked rows of x per batch, pad to a multiple of 128.  Padded
    rows are killed with an exp-bias of -30000 (exp -> 0), so they
    contribute to neither numerator nor denominator.
  * Scores are computed transposed, ST[m, n] = q_s[m].k[n], with m on PSUM
    partitions, so the exp's additive pad-penalty (per-m) is the scalar
    engine's per-partition bias operand, and E^T = exp(ST) lands in SBUF
    already in the right layout to be the matmul operand for both the
    numerator (x_c^T @ E^T -> out^T[d, n]) and the denominator
    (ones^T @ E^T -> den replicated over all 128 partitions, so the final
    divide is a plain elementwise op, no partition broadcast needed).
  * The 1/sqrt(DQ) scale is folded into Wq on the host (exact: 0.125 is a
    power of two).
  * All big matmuls run as float32r (full PE rate at free-dim >= 256),
    everything else fp32.
  * out^T is normalized, transposed back 128x128 via the PE, and DMA'd out.
"""

import math
import os

import numpy as np

B, N, D, DQ = 16, 2048, 128, 64
NCORES = 8
BPC = B // NCORES  # batches per core

_cache = {}


def _build_program(mcp: int, reps: int = 1):
    """Build the per-core Bass program for a compacted/padded m-size of mcp."""
    import concourse.bass as bass
    import concourse.tile as tile
    from concourse import bacc, mybir
    from concourse.masks import make_identity

    f32 = mybir.dt.float32
    f32r = mybir.dt.float32r
    mc = mcp // 128  # number of m chunks
    NG = 1024        # n-group width (2 PSUM banks)
    ngroups = N // NG

    nc = bacc.Bacc("TRN2", target_bir_lowering=False, debug=False, num_devices=1)

    xt = nc.dram_tensor("xt", [BPC, D, N], f32r, kind="ExternalInput").ap()
    xtc = nc.dram_tensor("xtc", [BPC, D, mcp], f32r, kind="ExternalInput").ap()
    xc = nc.dram_tensor("xc", [BPC, mcp, D], f32r, kind="ExternalInput").ap()
    pen = nc.dram_tensor("pen", [BPC, mcp, 1], f32, kind="ExternalInput").ap()
    wqt = nc.dram_tensor("wqt", [D, DQ], f32r, kind="ExternalInput").ap()
    wkt = nc.dram_tensor("wkt", [D, DQ], f32r, kind="ExternalInput").ap()
    bqs = nc.dram_tensor("bqs", [DQ, 1], f32, kind="ExternalInput").ap()
    bks = nc.dram_tensor("bks", [DQ, 1], f32, kind="ExternalInput").ap()
    onesd = nc.dram_tensor("onesd", [128, 128], f32r, kind="ExternalInput").ap()
    out = nc.dram_tensor("out", [BPC, N, D], f32, kind="ExternalOutput").ap()

    def r(ap):
        return ap

    with tile.TileContext(nc) as tc:
        with (
            tc.tile_pool(name="singles", bufs=1) as singles,
            tc.tile_pool(name="xtp", bufs=2) as xtp,
            tc.tile_pool(name="xtcp", bufs=2) as xtcp,
            tc.tile_pool(name="xcp", bufs=2 * mc) as xcp,
            tc.tile_pool(name="penp", bufs=2 * mc) as penp,
            tc.tile_pool(name="qtp", bufs=2) as qtp,
            tc.tile_pool(name="ktp", bufs=2) as ktp,
            tc.tile_pool(name="etp", bufs=3) as etp,
            tc.tile_pool(name="nrmp", bufs=2) as nrmp,
            tc.tile_pool(name="otp", bufs=4) as otp,
            tc.tile_pool(name="st", bufs=2, space="PSUM") as stp,
            tc.tile_pool(name="oa", bufs=1, space="PSUM") as oap,
            tc.tile_pool(name="dn", bufs=1, space="PSUM") as dnp,
        ):
            wq_sb = singles.tile([D, DQ], f32r)
            nc.sync.dma_start(wq_sb[:], wqt[:])
            wk_sb = singles.tile([D, DQ], f32r)
            nc.sync.dma_start(wk_sb[:], wkt[:])
            bq_sb = singles.tile([DQ, 1], f32)
            nc.sync.dma_start(bq_sb[:], bqs[:])
            bk_sb = singles.tile([DQ, 1], f32)
            nc.sync.dma_start(bk_sb[:], bks[:])
            ident = singles.tile([128, 128], f32)
            make_identity(nc, ident[:])
            ones = singles.tile([128, 128], f32r)
            nc.sync.dma_start(ones[:], onesd[:])

            for b in [b for _ in range(reps) for b in range(BPC)]:
                # ---- loads ----
                xt_t = xtp.tile([D, N], f32r)
                nc.sync.dma_start(xt_t[:], xt[b])
                xtc_t = xtcp.tile([D, mcp], f32r)
                nc.sync.dma_start(xtc_t[:], xtc[b])
                xc_t = []
                pen_t = []
                for m in range(mc):
                    t = xcp.tile([128, D], f32r, tag="xc")
                    nc.sync.dma_start(t[:], xc[b, m * 128:(m + 1) * 128, :])
                    xc_t.append(t)
                    p = penp.tile([128, 1], f32, tag="pen")
                    nc.sync.dma_start(p[:], pen[b, m * 128:(m + 1) * 128, :])
                    pen_t.append(p)

                # ---- projections (into [dq, m] / [dq, n] layout) ----
                qt_t = qtp.tile([DQ, mcp], f32r)
                for j in range(0, mcp, 512):
                    jw = min(512, mcp - j)
                    pp = stp.tile([128, NG], f32, tag="st")
                    nc.tensor.matmul(pp[0:DQ, 0:jw], r(wq_sb[:]),
                                     r(xtc_t[:, j:j + jw]), start=True, stop=True)
                    nc.vector.tensor_scalar_add(qt_t[:, j:j + jw], pp[0:DQ, 0:jw],
                                                bq_sb[:])
                kt_t = ktp.tile([DQ, N], f32r)
                for j in range(0, N, 512):
                    pp = stp.tile([128, NG], f32, tag="st")
                    nc.tensor.matmul(pp[0:DQ, 0:512], r(wk_sb[:]),
                                     r(xt_t[:, j:j + 512]), start=True, stop=True)
                    nc.vector.tensor_scalar_add(kt_t[:, j:j + 512], pp[0:DQ, 0:512],
                                                bk_sb[:])

                # ---- attention over n-groups ----
                nrm_tiles = []
                for g in range(ngroups):
                    oa = oap.tile([128, NG], f32, tag="oa")
                    dn = dnp.tile([128, NG], f32, tag="dn")
                    for m in range(mc):
                        st = stp.tile([128, NG], f32, tag="st")
                        for h in range(NG // 512):
                            nc.tensor.matmul(
                                st[:, h * 512:(h + 1) * 512],
                                r(qt_t[:, m * 128:(m + 1) * 128]),
                                r(kt_t[:, g * NG + h * 512: g * NG + (h + 1) * 512]),
                                start=True, stop=True)
                        et = etp.tile([128, NG], f32r, tag="et")
                        nc.scalar.activation(et[:], st[:],
                                             mybir.ActivationFunctionType.Exp,
                                             bias=pen_t[m][:], scale=1.0)
                        first, last = (m == 0), (m == mc - 1)
                        for h in range(NG // 512):
                            hs = slice(h * 512, (h + 1) * 512)
                            nc.tensor.matmul(oa[:, hs], r(xc_t[m][:]), r(et[:, hs]),
                                             start=first, stop=last)
                            nc.tensor.matmul(dn[:, hs], r(ones[:]), r(et[:, hs]),
                                             start=first, stop=last)
                    rden = nrmp.tile([128, NG], f32, tag="rden")
                    nc.vector.reciprocal(rden[:], dn[:])
                    nrm = nrmp.tile([128, NG], f32, tag="nrm")
                    nc.vector.tensor_mul(nrm[:], oa[:], rden[:])
                    nrm_tiles.append(nrm)

                # ---- transpose back and store ----
                for g in range(ngroups):
                    for j in range(NG // 128):
                        tp = stp.tile([128, NG], f32, tag="st")
                        nc.tensor.transpose(tp[:, 0:128],
                                            nrm_tiles[g][:, j * 128:(j + 1) * 128],
                                            ident[:])
                        ot = otp.tile([128, D], f32, tag="ot")
                        nc.vector.tensor_copy(ot[:], tp[:, 0:128])
                        n0 = g * NG + j * 128
                        nc.sync.dma_start(out[b, n0:n0 + 128, :], ot[:])

    nc.compile()
    return nc


def _prep(x, mask, Wq, bq, Wk, bk):
    """Host-side prep: compaction, transposes, sharding.  Returns (in_maps, mcp)."""
    x = np.ascontiguousarray(np.asarray(x, dtype=np.float32))
    mask = np.asarray(mask)
    Wq = np.asarray(Wq, dtype=np.float32)
    bq = np.asarray(bq, dtype=np.float32)
    Wk = np.asarray(Wk, dtype=np.float32)
    bk = np.asarray(bk, dtype=np.float32)

    scale = np.float32(1.0 / math.sqrt(DQ))

    # host-side compaction of the masked (aggregated) axis
    keep = [np.nonzero(mask[b] > 0)[0] for b in range(B)]
    counts = [len(k) for k in keep]
    mcap = max(max(counts), 1)
    mcp = ((mcap + 127) // 128) * 128

    xc = np.zeros((B, mcp, D), dtype=np.float32)
    pen = np.full((B, mcp, 1), -30000.0, dtype=np.float32)
    for b in range(B):
        cnt = counts[b]
        if cnt:
            xc[b, :cnt] = x[b, keep[b]]
            pen[b, :cnt] = 0.0

    xt = np.ascontiguousarray(x.transpose(0, 2, 1))          # [B, D, N]
    xtc = np.ascontiguousarray(xc.transpose(0, 2, 1))        # [B, D, mcp]
    wqt = np.ascontiguousarray((Wq * scale).T)               # [D, DQ]
    wkt = np.ascontiguousarray(Wk.T)                         # [D, DQ]
    bqs = np.ascontiguousarray((bq * scale).reshape(DQ, 1))
    bks = np.ascontiguousarray(bk.reshape(DQ, 1))
    ones_mat = np.ones((128, 128), dtype=np.float32)

    in_maps = []
    for c in range(NCORES):
        s = slice(c * BPC, (c + 1) * BPC)
        in_maps.append({
            "xt": xt[s], "xtc": xtc[s], "xc": xc[s], "pen": pen[s],
            "wqt": wqt, "wkt": wkt, "bqs": bqs, "bks": bks,
            "onesd": ones_mat,
        })
    return in_maps, mcp


def kernel(x, mask, Wq, bq, Wk, bk):
    from concourse import bass_utils

    in_maps, mcp = _prep(x, mask, Wq, bq, Wk, bk)

    if mcp not in _cache:
        _cache[mcp] = _build_program(mcp)
    nc = _cache[mcp]

    res = bass_utils.run_bass_kernel_spmd(
        nc, in_maps, core_ids=list(range(NCORES)),
        trace=bool(os.environ.get("BASS_TRACE")),
    )
    kernel._last_results = res

    out = np.concatenate([res.results[c]["out"] for c in range(NCORES)], axis=0)
    return out.astype(np.float32)


# revision 8
# speedup vs baseline: 6.1204x; 6.1204x over previous
"""Masked attention-aggregator kernel for Trainium2 (8 NeuronCores, SPMD).

Reference computation (B=16, N=2048, D=128, DQ=64), all fp32:
    q = x @ Wq.T + bq                      [B, N, DQ]
    k = x @ Wk.T + bk                      [B, N, DQ]
    s = (k @ q.T) / sqrt(DQ)               [B, N, N]   (s[b,n,m] = k[n].q[m])
    w = softmax(s + (mask[m]>0 ? 0 : -1e9), axis=m)
    out = w @ x                            [B, N, D]

Strategy: data-parallel over batch (2 batches per core).  Per batch, a
flash-style streaming attention that never materializes [N, N] in HBM:

  * The masked axis (m) indexes the *queries* side.  Masked m-columns get
    softmax weight exactly 0, so on the host we COMPACT the m axis: gather
    the unmasked rows of x per batch, pad to a multiple of 128.  Padded
    rows are killed with an exp-bias of -30000 (exp -> 0), so they
    contribute to neither numerator nor denominator.
  * Scores are computed transposed, ST[m, n] = q_s[m].k[n], with m on PSUM
    partitions, so the exp's additive pad-penalty (per-m) is the scalar
    engine's per-partition bias operand, and E^T = exp(ST) lands in SBUF
    already in the right layout to be the matmul operand for both the
    numerator (x_c^T @ E^T -> out^T[d, n]) and the denominator
    (ones^T @ E^T -> den replicated over all 128 partitions, so the final
    divide is a plain elementwise op, no partition broadcast needed).
  * The 1/sqrt(DQ) scale is folded into Wq on the host (exact: 0.125 is a
    power of two).
  * All big matmuls run as float32r (full PE rate at free-dim >= 256),
    everything else fp32.
  * out^T is normalized, transposed back 128x128 via the PE, and DMA'd out.
"""

import math
import os

import numpy as np

B, N, D, DQ = 16, 2048, 128, 64
NCORES = 8
BPC = B // NCORES  # batches per core

_cache = {}


def _build_program(mcp: int, reps: int = 1):
    """Build the per-core Bass program for a compacted/padded m-size of mcp."""
    import concourse.bass as bass
    import concourse.tile as tile
    from concourse import bacc, mybir
    from concourse.masks import make_identity

    f32 = mybir.dt.float32
    f32r = mybir.dt.float32r
    mc = mcp // 128  # number of m chunks
    NG = 1024        # n-group width (2 PSUM banks)
    ngroups = N // NG

    nc = bacc.Bacc("TRN2", target_bir_lowering=False, debug=False, num_devices=1)

    xt = nc.dram_tensor("xt", [BPC, D, N], f32r, kind="ExternalInput").ap()
    xtc = nc.dram_tensor("xtc", [BPC, D, mcp], f32r, kind="ExternalInput").ap()
    xc = nc.dram_tensor("xc", [BPC, mcp, D], f32r, kind="ExternalInput").ap()
    pen = nc.dram_tensor("pen", [BPC, mcp, 1], f32, kind="ExternalInput").ap()
    wqt = nc.dram_tensor("wqt", [D, DQ], f32r, kind="ExternalInput").ap()
    wkt = nc.dram_tensor("wkt", [D, DQ], f32r, kind="ExternalInput").ap()
    bqs = nc.dram_tensor("bqs", [DQ, 1], f32, kind="ExternalInput").ap()
    bks = nc.dram_tensor("bks", [DQ, 1], f32, kind="ExternalInput").ap()
    onesd = nc.dram_tensor("onesd", [128, 128], f32r, kind="ExternalInput").ap()
    out = nc.dram_tensor("out", [BPC, N, D], f32, kind="ExternalOutput").ap()

    def r(ap):
        return ap

    with tile.TileContext(nc) as tc:
        with (
            tc.tile_pool(name="singles", bufs=1) as singles,
            tc.tile_pool(name="xtp", bufs=2) as xtp,
            tc.tile_pool(name="xtcp", bufs=2) as xtcp,
            tc.tile_pool(name="xcp", bufs=2 * mc) as xcp,
            tc.tile_pool(name="penp", bufs=2 * mc) as penp,
            tc.tile_pool(name="qtp", bufs=2) as qtp,
            tc.tile_pool(name="ktp", bufs=2) as ktp,
            tc.tile_pool(name="etp", bufs=3) as etp,
            tc.tile_pool(name="nrmp", bufs=2) as nrmp,
            tc.tile_pool(name="otp", bufs=4) as otp,
            tc.tile_pool(name="st", bufs=2, space="PSUM") as stp,
            tc.tile_pool(name="oa", bufs=1, space="PSUM") as oap,
            tc.tile_pool(name="dn", bufs=1, space="PSUM") as dnp,
        ):
            wq_sb = singles.tile([D, DQ], f32r)
            nc.sync.dma_start(wq_sb[:], wqt[:])
            wk_sb = singles.tile([D, DQ], f32r)
            nc.sync.dma_start(wk_sb[:], wkt[:])
            bq_sb = singles.tile([DQ, 1], f32)
            nc.sync.dma_start(bq_sb[:], bqs[:])
            bk_sb = singles.tile([DQ, 1], f32)
            nc.sync.dma_start(bk_sb[:], bks[:])
            ident = singles.tile([128, 128], f32)
            make_identity(nc, ident[:])
            ones = singles.tile([128, 128], f32r)
            nc.sync.dma_start(ones[:], onesd[:])

            def body():
              for b in range(BPC):
                # ---- loads ----
                xt_t = xtp.tile([D, N], f32r)
                nc.sync.dma_start(xt_t[:], xt[b])
                xtc_t = xtcp.tile([D, mcp], f32r)
                nc.sync.dma_start(xtc_t[:], xtc[b])
                xc_t = []
                pen_t = []
                for m in range(mc):
                    t = xcp.tile([128, D], f32r, tag="xc")
                    nc.sync.dma_start(t[:], xc[b, m * 128:(m + 1) * 128, :])
                    xc_t.append(t)
                    p = penp.tile([128, 1], f32, tag="pen")
                    nc.sync.dma_start(p[:], pen[b, m * 128:(m + 1) * 128, :])
                    pen_t.append(p)

                # ---- projections (into [dq, m] / [dq, n] layout) ----
                qt_t = qtp.tile([DQ, mcp], f32r)
                for j in range(0, mcp, 512):
                    jw = min(512, mcp - j)
                    pp = stp.tile([128, NG], f32, tag="st")
                    nc.tensor.matmul(pp[0:DQ, 0:jw], r(wq_sb[:]),
                                     r(xtc_t[:, j:j + jw]), start=True, stop=True)
                    nc.vector.tensor_scalar_add(qt_t[:, j:j + jw], pp[0:DQ, 0:jw],
                                                bq_sb[:])
                kt_t = ktp.tile([DQ, N], f32r)
                for j in range(0, N, 512):
                    pp = stp.tile([128, NG], f32, tag="st")
                    nc.tensor.matmul(pp[0:DQ, 0:512], r(wk_sb[:]),
                                     r(xt_t[:, j:j + 512]), start=True, stop=True)
                    nc.vector.tensor_scalar_add(kt_t[:, j:j + 512], pp[0:DQ, 0:512],
                                                bk_sb[:])

                # ---- attention over n-groups ----
                nrm_tiles = []
                for g in range(ngroups):
                    oa = oap.tile([128, NG], f32, tag="oa")
                    dn = dnp.tile([128, NG], f32, tag="dn")
                    for m in range(mc):
                        st = stp.tile([128, NG], f32, tag="st")
                        for h in range(NG // 512):
                            nc.tensor.matmul(
                                st[:, h * 512:(h + 1) * 512],
                                r(qt_t[:, m * 128:(m + 1) * 128]),
                                r(kt_t[:, g * NG + h * 512: g * NG + (h + 1) * 512]),
                                start=True, stop=True)
                        et = etp.tile([128, NG], f32r, tag="et")
                        nc.scalar.activation(et[:], st[:],
                                             mybir.ActivationFunctionType.Exp,
                                             bias=pen_t[m][:], scale=1.0)
                        first, last = (m == 0), (m == mc - 1)
                        for h in range(NG // 512):
                            hs = slice(h * 512, (h + 1) * 512)
                            nc.tensor.matmul(oa[:, hs], r(xc_t[m][:]), r(et[:, hs]),
                                             start=first, stop=last)
                            nc.tensor.matmul(dn[:, hs], r(ones[:]), r(et[:, hs]),
                                             start=first, stop=last)
                    rden = nrmp.tile([128, NG], f32, tag="rden")
                    nc.vector.reciprocal(rden[:], dn[:])
                    nrm = nrmp.tile([128, NG], f32, tag="nrm")
                    nc.vector.tensor_mul(nrm[:], oa[:], rden[:])
                    nrm_tiles.append(nrm)

                # ---- transpose back and store ----
                for g in range(ngroups):
                    for j in range(NG // 128):
                        tp = stp.tile([128, NG], f32, tag="st")
                        nc.tensor.transpose(tp[:, 0:128],
                                            nrm_tiles[g][:, j * 128:(j + 1) * 128],
                                            ident[:])
                        ot = otp.tile([128, D], f32, tag="ot")
                        nc.vector.tensor_copy(ot[:], tp[:, 0:128])
                        n0 = g * NG + j * 128
                        nc.sync.dma_start(out[b, n0:n0 + 128, :], ot[:])

            if reps > 1:
                with tc.For_i(0, reps, 1):
                    body()
            else:
                body()

    nc.compile()
    return nc


def _prep(x, mask, Wq, bq, Wk, bk):
    """Host-side prep: compaction, transposes, sharding.  Returns (in_maps, mcp)."""
    x = np.ascontiguousarray(np.asarray(x, dtype=np.float32))
    mask = np.asarray(mask)
    Wq = np.asarray(Wq, dtype=np.float32)
    bq = np.asarray(bq, dtype=np.float32)
    Wk = np.asarray(Wk, dtype=np.float32)
    bk = np.asarray(bk, dtype=np.float32)

    scale = np.float32(1.0 / math.sqrt(DQ))

    # host-side compaction of the masked (aggregated) axis
    keep = [np.nonzero(mask[b] > 0)[0] for b in range(B)]
    counts = [len(k) for k in keep]
    mcap = max(max(counts), 1)
    mcp = ((mcap + 127) // 128) * 128

    xc = np.zeros((B, mcp, D), dtype=np.float32)
    pen = np.full((B, mcp, 1), -30000.0, dtype=np.float32)
    for b in range(B):
        cnt = counts[b]
        if cnt:
            xc[b, :cnt] = x[b, keep[b]]
            pen[b, :cnt] = 0.0

    xt = np.ascontiguousarray(x.transpose(0, 2, 1))          # [B, D, N]
    xtc = np.ascontiguousarray(xc.transpose(0, 2, 1))        # [B, D, mcp]
    wqt = np.ascontiguousarray((Wq * scale).T)               # [D, DQ]
    wkt = np.ascontiguousarray(Wk.T)                         # [D, DQ]
    bqs = np.ascontiguousarray((bq * scale).reshape(DQ, 1))
    bks = np.ascontiguousarray(bk.reshape(DQ, 1))
    ones_mat = np.ones((128, 128), dtype=np.float32)

    in_maps = []
    for c in range(NCORES):
        s = slice(c * BPC, (c + 1) * BPC)
        in_maps.append({
            "xt": xt[s], "xtc": xtc[s], "xc": xc[s], "pen": pen[s],
            "wqt": wqt, "wkt": wkt, "bqs": bqs, "bks": bks,
            "onesd": ones_mat,
        })
    return in_maps, mcp


def kernel(x, mask, Wq, bq, Wk, bk):
    from concourse import bass_utils

    in_maps, mcp = _prep(x, mask, Wq, bq, Wk, bk)

    if mcp not in _cache:
        _cache[mcp] = _build_program(mcp)
    nc = _cache[mcp]

    res = bass_utils.run_bass_kernel_spmd(
        nc, in_maps, core_ids=list(range(NCORES)),
        trace=bool(os.environ.get("BASS_TRACE")),
    )
    kernel._last_results = res

    out = np.concatenate([res.results[c]["out"] for c in range(NCORES)], axis=0)
    return out.astype(np.float32)
